# revision 1
# baseline (speedup 1.0000x reference)
"""Trainium2 Bass kernel for nn_Attn_Head (GNN attention head).

Computation (reference):
    seq_fts = x @ W1.T                      # [N, 64]
    f1 = seq_fts @ a1 ; f2 = seq_fts @ a2   # [N]
    logits[i, j] = leaky_relu(f1[j] + f2[i], 0.01)
    coefs = softmax(logits + bias_mx, axis=0)   # per-column softmax over i
    out = elu(coefs @ seq_fts)[None]        # [1, N, 64]

Sharding: columns j of the softmax matrix are block-sharded across the 8
NeuronCores (1024 columns each). The device works on the TRANSPOSED
matrix (tiles [j_partitions, i_free]) so that:
  - the softmax reduction over i runs along the free axis (fused into the
    Exp activation via accum_out),
  - the output matmul retT[c,i] = sum_j sf[j,c]*coefs[i,j] contracts over
    j on the partition axis with the small seq_fts block as the
    stationary operand (few LDWEIGHTS, 512-wide streaming).

leaky_relu is decomposed exactly as
    leaky(z) + bias = relu(0.99*z) + (bias + 0.01*(f1+f2))
where the rank-1 term 0.01*(f1[j]+f2[i]) is folded into the host-side
transposition pass of the bias matrix. Per-element device work is one
relu (split between ACT and DVE tensor_scalar to balance engines), one
add (DVE), one Exp+rowsum (ACT accum_out), plus the PE matmul. The
per-column softmax normalizers fold into the stationary seq_fts weights
(64 floats per partition) instead of rescaling the big matrix. Each core
emits a partial retT [64, 8192]; the host sums the 8 partials,
transposes, and applies the final elu.
"""

import sys

for _p in ("/opt/trn_rl_repo", "/root/.axon_site/_ro/trn_rl_repo"):
    if _p not in sys.path:
        sys.path.insert(0, _p)

import numpy as np

import concourse.bass as bass
import concourse.tile as tile
from concourse import mybir
from concourse.bass_utils import run_bass_kernel_spmd

N = 8192          # nodes
C = 256           # input channels
D = 64            # output size
NCORES = 8
B = N // NCORES   # columns per core (1024)
P = 128           # partitions
Q = B // P        # j-chunks per core (8)
HALF = N // 2     # i-subtile width (4096)
SEG = 512         # matmul streaming width
F32 = mybir.dt.float32


# ---------------------------------------------------------------------------
# Workaround: this walrus build rejects more than ONE sem-wait per
# instruction ("Too many sync wait commands"). After Tile lowering, split
# any instruction carrying k>1 waits into (k-1) single-wait NOPs on the
# same engine placed immediately before it — semantically identical, since
# an engine's sequencer processes waits in stream order.
def _split_multiwaits(nc):
    n_split = 0
    for f in nc.m.functions:
        for bb in f.blocks:
            insts = bb.instructions
            out = []
            for inst in insts:
                si = inst.sync_info
                if si is not None and si.on_wait and len(si.on_wait) > 1:
                    waits = list(si.on_wait)
                    for k, w in enumerate(waits[:-1]):
                        nop = mybir.InstNoOp(
                            name=f"{inst.name}.wsplit{k}", ins=[], outs=[]
                        )
                        nop.engine = inst.engine
                        nop.sync_info = mybir.SyncInfo(on_wait=[w], on_update=[])
                        out.append(nop)
                        n_split += 1
                    inst.sync_info = mybir.SyncInfo(
                        on_wait=[waits[-1]], on_update=list(si.on_update)
                    )
                out.append(inst)
            if len(out) != len(insts):
                bb.instructions = out
    return n_split
# ---------------------------------------------------------------------------


def build_nc(bias_bufs: int = 3, e_bufs: int = 3, t_bufs: int = 2,
             dve_relu: frozenset = frozenset({0, 2, 4, 6, 8, 10, 12, 14}),
             split_multiwaits: bool = True):
    """Build the per-core Bass program (SPMD: same program on all cores).

    dve_relu: set of tile indices (q*2+h) whose relu runs on the DVE
    (tensor_scalar) instead of the ACT engine — load balancing knob.
    """
    nc = bass.Bass("TRN2", target_bir_lowering=False, debug=False,
                   num_devices=NCORES)

    # Per-core inputs. f2v and f1s arrive pre-scaled by 0.99 (host).
    biasP = nc.dram_tensor("biasP", [B, N], F32, kind="ExternalInput")
    xT = nc.dram_tensor("xT", [C, B], F32, kind="ExternalInput")
    w1T = nc.dram_tensor("w1T", [C, D], F32, kind="ExternalInput")
    f1s = nc.dram_tensor("f1s", [P, Q], F32, kind="ExternalInput")
    f2v = nc.dram_tensor("f2v", [N], F32, kind="ExternalInput")
    ret = nc.dram_tensor("ret", [D, N], F32, kind="ExternalOutput")  # retT

    with tile.TileContext(nc) as tc:
        with (
            tc.tile_pool(name="singles", bufs=1) as singles,
            tc.tile_pool(name="bias", bufs=bias_bufs) as bias_pool,
            tc.tile_pool(name="t", bufs=t_bufs) as t_pool,
            tc.tile_pool(name="e", bufs=e_bufs) as e_pool,
            tc.tile_pool(name="psum", bufs=1, space="PSUM") as psum_pool,
        ):
            # --- setup: small loads -------------------------------------
            f1s_sb = singles.tile([P, Q], F32)
            nc.sync.dma_start(out=f1s_sb, in_=f1s[:, :])

            # q=0 bias halves first, then the 0.99*f2 broadcast to all
            # partitions (stride-0 DMA).
            bias_q0 = []
            for h in range(2):
                b_sb = bias_pool.tile([P, HALF], F32, tag="bias")
                nc.sync.dma_start(
                    out=b_sb, in_=biasP[0:P, h * HALF:(h + 1) * HALF]
                )
                bias_q0.append(b_sb)
            f2bc = singles.tile([P, N], F32)
            f2_b = bass.AP(tensor=f2v[:].tensor, offset=0, ap=[[0, P], [1, N]])
            nc.gpsimd.dma_start(out=f2bc, in_=f2_b)

            w1T_sb = singles.tile([P, 2, D], F32)      # k-chunks of W1.T
            nc.sync.dma_start(
                out=w1T_sb, in_=w1T[:, :].rearrange("(k p) d -> p k d", p=P)
            )
            xT_sb = singles.tile([P, 2, B], F32)       # k-chunks of x_blk.T
            nc.sync.dma_start(
                out=xT_sb, in_=xT[:, :].rearrange("(k p) b -> p k b", p=P)
            )

            # --- PSUM: retT [64, 8192] as 16 [64, 512] regions:
            # seg s<8  -> partitions 0:64,   bank s
            # seg s>=8 -> partitions 64:128, bank s-8
            ret_ps = psum_pool.tile([P, 8 * SEG], F32)

            def seg_out(s):
                if s < 8:
                    return ret_ps[0:D, s * SEG:(s + 1) * SEG], None
                return ret_ps[D:P, (s - 8) * SEG:(s - 7) * SEG], (0, 64)

            # --- seq_fts block: sf[jl, c] for this core's 1024 columns.
            # Borrows ret_ps[:, 0:64] before the main accumulation starts.
            sf_all = singles.tile([P, Q * D], F32)
            for qq in range(Q):
                for kc in range(2):
                    nc.tensor.matmul(
                        ret_ps[:, 0:D],
                        lhsT=xT_sb[:, kc, qq * P:(qq + 1) * P],
                        rhs=w1T_sb[:, kc, :],
                        start=(kc == 0),
                        stop=(kc == 1),
                    )
                nc.scalar.copy(out=sf_all[:, qq * D:(qq + 1) * D],
                               in_=ret_ps[:, 0:D])

            sf_scaled = singles.tile([P, Q * D], F32)
            s2 = singles.tile([P, 2 * Q], F32)     # per-half exp sums
            sq = singles.tile([P, Q], F32)         # row sums
            rinv = singles.tile([P, Q], F32)       # reciprocals
            # retT_sb[p,:]: p<64 -> retT[p, 0:4096]; p>=64 -> retT[p-64, 4096:]
            ret_sb = singles.tile([P, 8 * SEG], F32)

            # --- main loop over j-chunks --------------------------------
            for q in range(Q):
                # bias DMAs for both halves first (prefetch priority)
                bias_t = []
                for h in range(2):
                    if q == 0:
                        bias_t.append(bias_q0[h])
                    else:
                        b_sb = bias_pool.tile([P, HALF], F32, tag="bias")
                        nc.sync.dma_start(
                            out=b_sb,
                            in_=biasP[q * P:(q + 1) * P,
                                      h * HALF:(h + 1) * HALF],
                        )
                        bias_t.append(b_sb)

                e_halves = []
                for h in range(2):
                    # t = relu(0.99*f2[i] + 0.99*f1[jl])  (pre-scaled)
                    isl = slice(h * HALF, (h + 1) * HALF)
                    t_sb = t_pool.tile([P, HALF], F32, tag="t")
                    if (q * 2 + h) in dve_relu:
                        nc.vector.tensor_scalar(
                            out=t_sb, in0=f2bc[:, isl],
                            scalar1=f1s_sb[:, q:q + 1], scalar2=0.0,
                            op0=mybir.AluOpType.add, op1=mybir.AluOpType.max,
                        )
                    else:
                        nc.scalar.activation(
                            out=t_sb, in_=f2bc[:, isl],
                            func=mybir.ActivationFunctionType.Relu,
                            bias=f1s_sb[:, q:q + 1], scale=1.0,
                        )
                    # u = t + (biasT + 0.01*(f1+f2))   (in place into t)
                    nc.vector.tensor_add(t_sb, t_sb, bias_t[h])

                    # e = exp(u), accumulate row sums
                    e_sb = e_pool.tile([P, HALF], F32, tag="e")
                    nc.scalar.activation(
                        out=e_sb, in_=t_sb,
                        func=mybir.ActivationFunctionType.Exp,
                        accum_out=s2[:, 2 * q + h:2 * q + h + 1],
                    )
                    e_halves.append(e_sb)

                # normalizer -> fold into the stationary seq_fts weights
                nc.vector.tensor_add(sq[:, q:q + 1], s2[:, 2 * q:2 * q + 1],
                                     s2[:, 2 * q + 1:2 * q + 2])
                nc.vector.reciprocal(rinv[:, q:q + 1], sq[:, q:q + 1])
                nc.vector.tensor_scalar_mul(
                    sf_scaled[:, q * D:(q + 1) * D],
                    sf_all[:, q * D:(q + 1) * D],
                    rinv[:, q:q + 1],
                )

                # retT[seg] += sf_scaled[q].T @ e[seg]   (sf stationary).
                # On the last q, evacuate each PSUM segment right after its
                # final matmul and kick the output DMA per partition-half.
                for s in range(16):
                    h, sl = divmod(s, 8)
                    out_ap, tpos = seg_out(s)
                    nc.tensor.matmul(
                        out_ap,
                        lhsT=sf_scaled[:, q * D:(q + 1) * D],
                        rhs=e_halves[h][:, sl * SEG:(sl + 1) * SEG],
                        start=(q == 0),
                        stop=(q == Q - 1),
                        tile_position=tpos,
                    )
                    if q == Q - 1:
                        dst = (ret_sb[0:D, s * SEG:(s + 1) * SEG] if s < 8
                               else ret_sb[D:P, (s - 8) * SEG:(s - 7) * SEG])
                        if s % 2 == 0:
                            nc.scalar.copy(out=dst, in_=out_ap)
                        else:
                            nc.vector.tensor_copy(dst, out_ap)
                        if s == 7:
                            nc.sync.dma_start(out=ret[:, 0:HALF],
                                              in_=ret_sb[0:D, :])
                        elif s == 15:
                            nc.sync.dma_start(out=ret[:, HALF:N],
                                              in_=ret_sb[D:P, :])

    if split_multiwaits:
        _split_multiwaits(nc)
    return nc


_NC_CACHE = None


def _get_nc():
    global _NC_CACHE
    if _NC_CACHE is None:
        _NC_CACHE = build_nc()
    return _NC_CACHE


def host_prep(x, bias_mx, W1, a1, a2):
    """Shard + lay out inputs for the 8 cores (f32 throughout)."""
    x = np.ascontiguousarray(x, dtype=np.float32)
    W1 = np.ascontiguousarray(W1, dtype=np.float32)
    sf_host = x @ W1.T                   # only used for f1/f2 (rank-1 term)
    f1 = sf_host @ np.asarray(a1, dtype=np.float32)
    f2 = sf_host @ np.asarray(a2, dtype=np.float32)

    w1T = np.ascontiguousarray(W1.T)
    f2s = np.ascontiguousarray(0.99 * f2)
    in_maps = []
    for d in range(NCORES):
        j0 = d * B
        blk = bias_mx[:, j0:j0 + B]
        biasP = np.empty((B, N), dtype=np.float32)
        np.copyto(biasP, blk.T)
        biasP += (0.01 * f1[j0:j0 + B])[:, None]
        biasP += (0.01 * f2)[None, :]
        in_maps.append({
            "biasP": biasP,
            "xT": np.ascontiguousarray(x[j0:j0 + B].T),
            "w1T": w1T,
            "f1s": np.ascontiguousarray(
                (0.99 * f1[j0:j0 + B]).reshape(Q, P).T
            ),
            "f2v": f2s,
        })
    return in_maps


def postprocess(results):
    retT = results[0]["ret"].astype(np.float32)
    for d in range(1, NCORES):
        retT = retT + results[d]["ret"]
    r = retT.T
    out = np.where(r > 0.0, r, np.expm1(np.minimum(r, 0.0)))
    return np.ascontiguousarray(out[None], dtype=np.float32)


def kernel(x, bias_mx, W1, a1, a2):
    nc = _get_nc()
    in_maps = host_prep(x, bias_mx, W1, a1, a2)
    res = run_bass_kernel_spmd(nc, in_maps, list(range(NCORES)))
    return postprocess(res.results)


if __name__ == "__main__":
    rng = np.random.default_rng(0)
    x = rng.standard_normal((N, C), dtype=np.float32)
    bias_mx = rng.standard_normal((N, N), dtype=np.float32)
    W1 = rng.standard_normal((D, C), dtype=np.float32) / np.sqrt(C)
    a1 = rng.standard_normal(D).astype(np.float32) / np.sqrt(D)
    a2 = rng.standard_normal(D).astype(np.float32) / np.sqrt(D)
    out = kernel(x=x, bias_mx=bias_mx, W1=W1, a1=a1, a2=a2)
    print("out", out.shape, out.dtype, float(np.abs(out).max()))



# revision 3
# speedup vs baseline: 1.9168x; 1.9168x over previous
"""Trainium2 Bass kernel for nn_Attn_Head (GNN attention head).

Computation (reference):
    seq_fts = x @ W1.T                      # [N, 64]
    f1 = seq_fts @ a1 ; f2 = seq_fts @ a2   # [N]
    logits[i, j] = leaky_relu(f1[j] + f2[i], 0.01)
    coefs = softmax(logits + bias_mx, axis=0)   # per-column softmax over i
    out = elu(coefs @ seq_fts)[None]        # [1, N, 64]

Sharding: columns j of the softmax matrix are block-sharded across the 8
NeuronCores (1024 columns each). The device works on the TRANSPOSED
matrix (tiles [j_partitions, i_free]) so that:
  - the softmax reduction over i runs along the free axis (fused into the
    Exp activation via accum_out),
  - the output matmul retT[c,i] = sum_j sf[j,c]*coefs[i,j] contracts over
    j on the partition axis with the small seq_fts block as the
    stationary operand (few LDWEIGHTS, 512-wide streaming).

leaky_relu is decomposed exactly as
    leaky(z) + bias = relu(0.99*z) + (bias + 0.01*(f1+f2))
where the rank-1 term 0.01*(f1[j]+f2[i]) is folded into the host-side
transposition pass of the bias matrix.

Precision strategy (hw rel err ~1.8e-3, gate 2e-2): the bias matrix is
shipped fp16 (halves the dominant HBM stream) and the elementwise chain
runs fp16 end-to-end, which puts the DVE in its 4x (tensor_scalar) /
2x (tensor_tensor) packed perf modes. The Exp emits bf16 (values up to
~2e5 exceed fp16 range) with the row sums accumulated in fp32, and both
matmul operands are bf16, which runs the PE at full rate instead of
fp32's two-pass half-rate mode. PSUM accumulation stays fp32, the
per-column normalizers fold into the stationary seq_fts weights, and
the partial retT [64, 8192] leaves each core fp32; the host sums the 8
partials, transposes, and applies the final elu.
"""

import sys

for _p in ("/opt/trn_rl_repo", "/root/.axon_site/_ro/trn_rl_repo"):
    if _p not in sys.path:
        sys.path.insert(0, _p)

import numpy as np
import ml_dtypes

import concourse.bass as bass
import concourse.tile as tile
from concourse import mybir
from concourse.bass_utils import run_bass_kernel_spmd

N = 8192          # nodes
C = 256           # input channels
D = 64            # output size
NCORES = 8
B = N // NCORES   # columns per core (1024)
P = 128           # partitions
Q = B // P        # j-chunks per core (8)
HALF = N // 2     # i-subtile width (4096)
SEG = 512         # matmul streaming width
F32 = mybir.dt.float32
F16 = mybir.dt.float16
BF16 = mybir.dt.bfloat16
NP_BF16 = ml_dtypes.bfloat16


# ---------------------------------------------------------------------------
# Workaround: this walrus build rejects more than ONE sem-wait per
# instruction ("Too many sync wait commands"). After Tile lowering, split
# any instruction carrying k>1 waits into (k-1) single-wait NOPs on the
# same engine placed immediately before it — semantically identical, since
# an engine's sequencer processes waits in stream order.
def _split_multiwaits(nc):
    n_split = 0
    for f in nc.m.functions:
        for bb in f.blocks:
            insts = bb.instructions
            out = []
            for inst in insts:
                si = inst.sync_info
                if si is not None and si.on_wait and len(si.on_wait) > 1:
                    waits = list(si.on_wait)
                    for k, w in enumerate(waits[:-1]):
                        nop = mybir.InstNoOp(
                            name=f"{inst.name}.wsplit{k}", ins=[], outs=[]
                        )
                        nop.engine = inst.engine
                        nop.sync_info = mybir.SyncInfo(on_wait=[w], on_update=[])
                        out.append(nop)
                        n_split += 1
                    inst.sync_info = mybir.SyncInfo(
                        on_wait=[waits[-1]], on_update=list(si.on_update)
                    )
                out.append(inst)
            if len(out) != len(insts):
                bb.instructions = out
    return n_split
# ---------------------------------------------------------------------------


def build_nc(bias_bufs: int = 4, e_bufs: int = 2, t_bufs: int = 2,
             split_multiwaits: bool = True):
    """Build the per-core Bass program (SPMD: same program on all cores)."""
    nc = bass.Bass("TRN2", target_bir_lowering=False, debug=False,
                   num_devices=NCORES)

    # Per-core inputs. f2v and f1s arrive pre-scaled by 0.99 (host).
    biasP = nc.dram_tensor("biasP", [B, N], F16, kind="ExternalInput")
    xT = nc.dram_tensor("xT", [C, B], BF16, kind="ExternalInput")
    w1T = nc.dram_tensor("w1T", [C, D], BF16, kind="ExternalInput")
    f1s = nc.dram_tensor("f1s", [P, Q], F32, kind="ExternalInput")
    f2v = nc.dram_tensor("f2v", [N], F16, kind="ExternalInput")
    ret = nc.dram_tensor("ret", [D, N], F32, kind="ExternalOutput")  # retT

    with tile.TileContext(nc) as tc:
        with (
            tc.tile_pool(name="singles", bufs=1) as singles,
            tc.tile_pool(name="bias", bufs=bias_bufs) as bias_pool,
            tc.tile_pool(name="t", bufs=t_bufs) as t_pool,
            tc.tile_pool(name="e", bufs=e_bufs) as e_pool,
            tc.tile_pool(name="psum", bufs=1, space="PSUM") as psum_pool,
        ):
            # --- setup: small loads -------------------------------------
            f1s_sb = singles.tile([P, Q], F32)
            nc.sync.dma_start(out=f1s_sb, in_=f1s[:, :])

            # q=0 bias halves first, then the 0.99*f2 broadcast to all
            # partitions (stride-0 DMA).
            bias_q0 = []
            for h in range(2):
                b_sb = bias_pool.tile([P, HALF], F16, tag="bias")
                nc.sync.dma_start(
                    out=b_sb, in_=biasP[0:P, h * HALF:(h + 1) * HALF]
                )
                bias_q0.append(b_sb)
            f2bc = singles.tile([P, N], F16)
            f2_b = bass.AP(tensor=f2v[:].tensor, offset=0, ap=[[0, P], [1, N]])
            nc.gpsimd.dma_start(out=f2bc, in_=f2_b)

            w1T_sb = singles.tile([P, 2, D], BF16)     # k-chunks of W1.T
            nc.sync.dma_start(
                out=w1T_sb, in_=w1T[:, :].rearrange("(k p) d -> p k d", p=P)
            )
            xT_sb = singles.tile([P, 2, B], BF16)      # k-chunks of x_blk.T
            nc.sync.dma_start(
                out=xT_sb, in_=xT[:, :].rearrange("(k p) b -> p k b", p=P)
            )

            # --- PSUM: retT [64, 8192] as 16 [64, 512] regions:
            # seg s<8  -> partitions 0:64,   bank s
            # seg s>=8 -> partitions 64:128, bank s-8
            ret_ps = psum_pool.tile([P, 8 * SEG], F32)

            def seg_out(s):
                if s < 8:
                    return ret_ps[0:D, s * SEG:(s + 1) * SEG], None
                return ret_ps[D:P, (s - 8) * SEG:(s - 7) * SEG], (0, 64)

            # --- seq_fts block: sf[jl, c] for this core's 1024 columns.
            # Borrows ret_ps[:, 0:64] before the main accumulation starts.
            sf_all = singles.tile([P, Q * D], F32)
            for qq in range(Q):
                for kc in range(2):
                    nc.tensor.matmul(
                        ret_ps[:, 0:D],
                        lhsT=xT_sb[:, kc, qq * P:(qq + 1) * P],
                        rhs=w1T_sb[:, kc, :],
                        start=(kc == 0),
                        stop=(kc == 1),
                    )
                nc.scalar.copy(out=sf_all[:, qq * D:(qq + 1) * D],
                               in_=ret_ps[:, 0:D])

            sf_scaled = singles.tile([P, Q * D], BF16)
            sq = singles.tile([P, Q], F32)         # row sums
            rinv = singles.tile([P, Q], F32)       # reciprocals
            # retT_sb[p,:]: p<64 -> retT[p, 0:4096]; p>=64 -> retT[p-64, 4096:]
            ret_sb = singles.tile([P, 8 * SEG], F32)

            # --- main loop over j-chunks --------------------------------
            for q in range(Q):
                # bias DMAs for both halves first (prefetch priority)
                bias_t = []
                for h in range(2):
                    if q == 0:
                        bias_t.append(bias_q0[h])
                    else:
                        b_sb = bias_pool.tile([P, HALF], F16, tag="bias")
                        nc.sync.dma_start(
                            out=b_sb,
                            in_=biasP[q * P:(q + 1) * P,
                                      h * HALF:(h + 1) * HALF],
                        )
                        bias_t.append(b_sb)

                # t = relu(0.99*f2[i] + 0.99*f1[jl])  (pre-scaled), full row
                t_sb = t_pool.tile([P, N], F16, tag="t")
                nc.vector.tensor_scalar(
                    out=t_sb, in0=f2bc,
                    scalar1=f1s_sb[:, q:q + 1], scalar2=0.0,
                    op0=mybir.AluOpType.add, op1=mybir.AluOpType.max,
                )
                # u = t + (biasT + 0.01*(f1+f2))   (in place, per half)
                for h in range(2):
                    isl = slice(h * HALF, (h + 1) * HALF)
                    nc.vector.tensor_add(t_sb[:, isl], t_sb[:, isl], bias_t[h])

                # e = exp(u) (bf16), row sum accumulated in fp32
                e_sb = e_pool.tile([P, N], BF16, tag="e")
                nc.scalar.activation(
                    out=e_sb, in_=t_sb,
                    func=mybir.ActivationFunctionType.Exp,
                    accum_out=sq[:, q:q + 1],
                )

                # normalizer -> fold into the stationary seq_fts weights
                nc.vector.reciprocal(rinv[:, q:q + 1], sq[:, q:q + 1])
                nc.vector.tensor_scalar_mul(
                    sf_scaled[:, q * D:(q + 1) * D],
                    sf_all[:, q * D:(q + 1) * D],
                    rinv[:, q:q + 1],
                )

                # retT[seg] += sf_scaled[q].T @ e[seg]   (sf stationary).
                # On the last q, evacuate each PSUM segment right after its
                # final matmul and kick the output DMA per partition-half.
                for s in range(16):
                    out_ap, tpos = seg_out(s)
                    nc.tensor.matmul(
                        out_ap,
                        lhsT=sf_scaled[:, q * D:(q + 1) * D],
                        rhs=e_sb[:, s * SEG:(s + 1) * SEG],
                        start=(q == 0),
                        stop=(q == Q - 1),
                        tile_position=tpos,
                    )
                    if q == Q - 1:
                        dst = (ret_sb[0:D, s * SEG:(s + 1) * SEG] if s < 8
                               else ret_sb[D:P, (s - 8) * SEG:(s - 7) * SEG])
                        if s % 2 == 0:
                            nc.scalar.copy(out=dst, in_=out_ap)
                        else:
                            nc.vector.tensor_copy(dst, out_ap)
                        if s == 7:
                            nc.sync.dma_start(out=ret[:, 0:HALF],
                                              in_=ret_sb[0:D, :])
                        elif s == 15:
                            nc.sync.dma_start(out=ret[:, HALF:N],
                                              in_=ret_sb[D:P, :])

    if split_multiwaits:
        _split_multiwaits(nc)
    return nc


_NC_CACHE = None


def _get_nc():
    global _NC_CACHE
    if _NC_CACHE is None:
        _NC_CACHE = build_nc()
    return _NC_CACHE


def host_prep(x, bias_mx, W1, a1, a2):
    """Shard + lay out inputs for the 8 cores."""
    x = np.ascontiguousarray(x, dtype=np.float32)
    W1 = np.ascontiguousarray(W1, dtype=np.float32)
    sf_host = x @ W1.T                   # only used for f1/f2 (rank-1 term)
    f1 = sf_host @ np.asarray(a1, dtype=np.float32)
    f2 = sf_host @ np.asarray(a2, dtype=np.float32)

    w1T = np.ascontiguousarray(W1.T.astype(NP_BF16))
    f2s = np.ascontiguousarray((0.99 * f2).astype(np.float16))
    in_maps = []
    for d in range(NCORES):
        j0 = d * B
        blk = bias_mx[:, j0:j0 + B]
        biasP = np.empty((B, N), dtype=np.float32)
        np.copyto(biasP, blk.T)
        biasP += (0.01 * f1[j0:j0 + B])[:, None]
        biasP += (0.01 * f2)[None, :]
        in_maps.append({
            "biasP": biasP.astype(np.float16),
            "xT": np.ascontiguousarray(x[j0:j0 + B].T.astype(NP_BF16)),
            "w1T": w1T,
            "f1s": np.ascontiguousarray(
                (0.99 * f1[j0:j0 + B]).reshape(Q, P).T
            ),
            "f2v": f2s,
        })
    return in_maps


def postprocess(results):
    retT = results[0]["ret"].astype(np.float32)
    for d in range(1, NCORES):
        retT = retT + results[d]["ret"]
    r = retT.T
    out = np.where(r > 0.0, r, np.expm1(np.minimum(r, 0.0)))
    return np.ascontiguousarray(out[None], dtype=np.float32)


def kernel(x, bias_mx, W1, a1, a2):
    nc = _get_nc()
    in_maps = host_prep(x, bias_mx, W1, a1, a2)
    res = run_bass_kernel_spmd(nc, in_maps, list(range(NCORES)))
    return postprocess(res.results)


if __name__ == "__main__":
    rng = np.random.default_rng(0)
    x = rng.standard_normal((N, C), dtype=np.float32)
    bias_mx = rng.standard_normal((N, N), dtype=np.float32)
    W1 = rng.standard_normal((D, C), dtype=np.float32) / np.sqrt(C)
    a1 = rng.standard_normal(D).astype(np.float32) / np.sqrt(D)
    a2 = rng.standard_normal(D).astype(np.float32) / np.sqrt(D)
    out = kernel(x=x, bias_mx=bias_mx, W1=W1, a1=a1, a2=a2)
    print("out", out.shape, out.dtype, float(np.abs(out).max()))
